# revision 63
# baseline (speedup 1.0000x reference)
"""Trainium2 Bass kernel for nn_CA_2568390443063.

PoolBlock (2x depthwise-conv3x3-s2 + BN + PReLU) -> channel-similarity
softmax -> out = sim^T @ x, data-parallel over batch (1 sample / core,
8 NeuronCores).

Per-core plan (channels C=128 live on the 128 SBUF partitions):
  Phase 1: stream x (128,256,256) fp32 from HBM in 8-row groups
    (1 MB DMAs alternating between the SP/HWDGE and GpSimd/SWDGE rings)
    into rotating fp32 buffers, then cast each group into ONE padded fp16
    x-cache (258x260 per partition) that serves both the conv taps and
    the phase-3 matmul — fp16's 10-bit mantissa keeps rounding noise at
    the ~1e-3 level while halving storage so the whole sample fits in
    SBUF (no second HBM read of x). conv1 runs on the TensorEngine as 9
    accumulating diagonal matmuls (BN scale folded into the diag
    weights on host); epilogue = ACT affine(+BN bias) + DVE max(y, a*y)
    (PReLU) into a sliding 17-row padded fp16 h1 window. conv2 runs
    identically off the window, emitted one group late so the PE never
    waits on the window epilogue; each 128-col h2 tile is PE-transposed
    and staged.
  Phase 2: S = pf @ pf^T accumulated over the 32 transposed tiles
    (fp16 matmuls, fp32 PSUM); softmax along free dim via DVE max /
    ACT exp(accum_out=rowsum) / DVE reciprocal+scale -> sim (fp16).
  Phase 3: out = sim^T @ x as 128 fp16 N=512 matmuls (1 cyc/row)
    straight out of the fp16 cache into paired PSUM banks; one
    1024-elem PSUM->SBUF copy per 4-row group, alternating ACT/DVE;
    512 KB out-DMAs alternate the SP and GpSimd/SWDGE rings. The
    similarity accumulation is interleaved into phase 1 (one matmul
    per transposed tile, own PSUM bank), so only the softmax sits
    between the phases.

Numerics: fp16 conv/similarity/final path, fp32 PSUM accumulation and
fp32 output. Measured ~4e-4 rel error vs the fp32 reference.
"""
import sys
import numpy as np

sys.path.insert(0, "/opt/trn_rl_repo")

import concourse.tile as tile  # noqa: E402
from concourse import bacc, mybir  # noqa: E402

EPS = 1e-5
P = 128          # channels == SBUF partitions
H = W = 256
H1 = W1 = 128    # after conv1 (stride 2)
H2 = W2 = 64     # after conv2
N2 = H2 * W2     # 4096
NX = H * W       # 65536
WR = W + 4       # padded x-cache row stride (260); data cols at 2..257
HR = W1 + 4      # h1 window row stride (132); data cols at 2..129
TAPS = [(dy, dx) for dy in (-1, 0, 1) for dx in (-1, 0, 1)]

f32 = mybir.dt.float32
f16 = mybir.dt.float16
AF = mybir.ActivationFunctionType
ALU = mybir.AluOpType


def build_nc(n_iters: int = 1, internal_io: bool = False):
    """Build the per-core Bass program (identical on all 8 cores).

    internal_io=True replaces the big x/out tensors with internal DRAM
    (zero-filled once) and adds a tiny external out — a timing-only
    variant whose per-call wall time is not dominated by host transfers.
    """
    nc = bacc.Bacc("TRN2", target_bir_lowering=False, debug=False,
                   enable_asserts=True, num_devices=8)

    w1_d = nc.dram_tensor("w1d", [P, 9 * P], f16, kind="ExternalInput")
    w2_d = nc.dram_tensor("w2d", [P, 9 * P], f16, kind="ExternalInput")
    prm_d = nc.dram_tensor("prm", [P, 4], f32, kind="ExternalInput")
    idn_d = nc.dram_tensor("idn", [P, P], f16, kind="ExternalInput")
    if internal_io:
        x_d = nc.dram_tensor("xint", [P, H, W], f32)
        out_d = nc.dram_tensor("oint", [P, NX], f32)
        small_d = nc.dram_tensor("out", [P, 4], f32, kind="ExternalOutput")
    else:
        x_d = nc.dram_tensor("x", [P, H, W], f32, kind="ExternalInput")
        out_d = nc.dram_tensor("out", [P, NX], f32, kind="ExternalOutput")
        small_d = None

    with tile.TileContext(nc) as tc:
        if internal_io:
            with tc.tile_pool(name="zf", bufs=1) as zf:
                z = zf.tile([P, 8 * W], f32)
                nc.vector.memset(z[:, :], 0.0)
                for m in range(32):
                    nc.sync.dma_start(out=x_d[:, 8 * m:8 * m + 8, :],
                                      in_=z[:, :])
        _emit(nc, tc, x_d, w1_d, w2_d, prm_d, idn_d, out_d, n_iters)
        if small_d is not None:
            with tc.tile_pool(name="smo", bufs=1) as smo:
                t = smo.tile([P, 4], f32)
                nc.sync.dma_start(out=t[:, :], in_=x_d[:, 0, 0:4])
                nc.sync.dma_start(out=small_d[:, :], in_=t[:, :])
    nc.compile()  # bacc register allocation / DCE
    return nc


def _emit(nc, tc, x_d, w1_d, w2_d, prm_d, idn_d, out_d, n_iters):
    from contextlib import ExitStack

    with ExitStack() as ctx:
        ep = ctx.enter_context
        consts = ep(tc.tile_pool(name="consts", bufs=1))
        cachep = ep(tc.tile_pool(name="cache", bufs=1))
        xf32p = ep(tc.tile_pool(name="xf32", bufs=3))
        ptp = ep(tc.tile_pool(name="pt", bufs=6))
        ostp = ep(tc.tile_pool(name="ost", bufs=6))
        smallp = ep(tc.tile_pool(name="small", bufs=2))
        spsum = ep(tc.tile_pool(name="spsum", bufs=1, space="PSUM"))

        # ---- preload constants
        w1sb = consts.tile([P, 9, P], f16)
        w2sb = consts.tile([P, 9, P], f16)
        prm = consts.tile([P, 4], f32)
        idn = consts.tile([P, P], f16)
        nc.sync.dma_start(out=w1sb[:, :, :], in_=w1_d[:, :].rearrange(
            "p (t c) -> p t c", c=P))
        nc.sync.dma_start(out=w2sb[:, :, :], in_=w2_d[:, :].rearrange(
            "p (t c) -> p t c", c=P))
        nc.sync.dma_start(out=prm[:, :], in_=prm_d[:, :])
        nc.sync.dma_start(out=idn[:, :], in_=idn_d[:, :])
        b1 = prm[:, 0:1]
        a1 = prm[:, 1:2]
        b2 = prm[:, 2:3]
        a2 = prm[:, 3:4]

        # Padded fp16 x cache: x row r at cache row r+1, col c at c+2.
        # Row 258 is a spare so strided tap slices may have stop=258.
        xch = cachep.tile([P, 258, WR], f16)
        nc.gpsimd.memset(xch[:, 0, :], 0.0)     # top pad row (x row -1)
        nc.gpsimd.memset(xch[:, :, 0:2], 0.0)   # left pad columns

        for _ in range(n_iters):
            _emit_iter(nc, tc, x_d, out_d, w1sb, w2sb, idn,
                       b1, a1, b2, a2, xch, xf32p, ptp, ostp,
                       smallp, spsum)


def _emit_iter(nc, tc, x_d, out_d, w1sb, w2sb, idn, b1, a1, b2, a2,
               xch, xf32p, ptp, ostp, smallp, spsum):
    from contextlib import ExitStack

    pts = []
    with ExitStack() as phase1:
        ybufp = phase1.enter_context(tc.tile_pool(name="ybuf", bufs=3))
        h1wp = phase1.enter_context(tc.tile_pool(name="h1w", bufs=3))
        h2p = phase1.enter_context(tc.tile_pool(name="h2b", bufs=2))
        cpsum = phase1.enter_context(
            tc.tile_pool(name="cpsum", bufs=2, space="PSUM"))
        tpsum = phase1.enter_context(
            tc.tile_pool(name="tpsum", bufs=2, space="PSUM"))

        def new_window(first):
            # h1 sliding window: rows 16*g2-1 .. 16*g2+15 at local 0..16
            # (18 rows: 17 used + spare for strided slice stop)
            w_ = h1wp.tile([P, 18, HR], f16, tag="h1w")
            nc.gpsimd.memset(w_[:, :, 0:2], 0.0)  # left halo column
            if first:
                nc.gpsimd.memset(w_[:, 0, :], 0.0)  # top pad row (-1)
            return w_

        S = spsum.tile([P, P], f32)
        wins = {0: new_window(first=True)}
        for m in range(32):
            r0 = 8 * m
            # ---- land 8 fp32 rows, cast into the padded fp16 cache
            xl = xf32p.tile([P, 8 * W], f32)
            eng = (nc.sync, nc.gpsimd)[m % 2]
            if m == 0:
                # split the first group so the conv pipeline ramps sooner
                nc.sync.dma_start(out=xl[:, :4 * W], in_=x_d[:, 0:4, :])
                nc.gpsimd.dma_start(out=xl[:, 4 * W:], in_=x_d[:, 4:8, :])
                nc.vector.tensor_copy(
                    xch[:, 1:5, 2:2 + W],
                    xl[:, :4 * W].rearrange("p (a b) -> p a b", b=W))
                nc.vector.tensor_copy(
                    xch[:, 5:9, 2:2 + W],
                    xl[:, 4 * W:].rearrange("p (a b) -> p a b", b=W))
            else:
                eng.dma_start(out=xl[:, :], in_=x_d[:, r0:r0 + 8, :])
                nc.vector.tensor_copy(
                    xch[:, r0 + 1:r0 + 9, 2:2 + W],
                    xl[:, :].rearrange("p (a b) -> p a b", b=W))

            # ---- conv1 block: output rows 4m..4m+3 (9 taps, diag lhsT)
            c1 = cpsum.tile([P, 4, W1], f32, tag="cps")
            for t, (dy, dx) in enumerate(TAPS):
                rhs = xch[:, r0 + dy + 1:r0 + dy + 1 + 8:2,
                          2 + dx:2 + dx + 2 * W1:2]
                nc.tensor.matmul(c1[:, :, :], w1sb[:, t, :], rhs,
                                 start=(t == 0), stop=(t == 8))
            y = ybufp.tile([P, 4, W1], f16, tag="yb")
            nc.scalar.activation(y[:, :, :], c1[:, :, :], AF.Identity,
                                 bias=b1, scale=1.0)
            loc = 4 * (m % 4) + 1  # local row of h1 row 4m in window m//4
            nc.vector.scalar_tensor_tensor(
                out=wins[m // 4][:, loc:loc + 4, 2:2 + W1],
                in0=y[:, :, :], scalar=a1, in1=y[:, :, :],
                op0=ALU.mult, op1=ALU.max)
            if m % 4 == 3 and m < 31:
                # last h1 row of this window is also row -1 of the next
                wins[m // 4 + 1] = new_window(first=False)
                nc.vector.scalar_tensor_tensor(
                    out=wins[m // 4 + 1][:, 0:1, 2:2 + W1],
                    in0=y[:, 3:4, :], scalar=a1, in1=y[:, 3:4, :],
                    op0=ALU.mult, op1=ALU.max)

            # ---- conv2 block g2, emitted one group after its window
            # completes so the PE has independent conv1 work while the
            # window epilogue (ACT+DVE) drains
            if m >= 4 and m % 4 == 0:
                g2 = (m - 4) // 4
                _emit_conv2(nc, g2, wins.pop(g2), w2sb, idn, b2, a2,
                            ybufp, h2p, cpsum, tpsum, ptp, pts, S)

        _emit_conv2(nc, 7, wins.pop(7), w2sb, idn, b2, a2,
                    ybufp, h2p, cpsum, tpsum, ptp, pts, S)

    # ---- softmax over free dim, scaled by N2^-0.5
    scale = float(N2) ** -0.5
    mx = smallp.tile([P, 1], f32, tag="sm")
    mb = smallp.tile([P, 1], f32, tag="sm")
    den = smallp.tile([P, 1], f32, tag="sm")
    rcp = smallp.tile([P, 1], f32, tag="sm")
    E = smallp.tile([P, P], f32, tag="esm")
    sim = smallp.tile([P, P], f16, tag="simt")
    nc.vector.reduce_max(mx[:, :], S[:, :], axis=mybir.AxisListType.X)
    nc.vector.tensor_scalar_mul(mb[:, :], mx[:, :], -scale)
    nc.scalar.activation(E[:, :], S[:, :], AF.Exp, bias=mb[:, :],
                         scale=scale, accum_out=den[:, :])
    nc.vector.reciprocal(rcp[:, :], den[:, :])
    nc.vector.tensor_scalar_mul(sim[:, :], E[:, :], rcp[:, :])

    # ---- out = sim^T @ x: 128 fp16 N=512 matmuls off the fp16 cache.
    # 64 output groups of 4 rows; short ost lifetimes + 3 DMA rings keep
    # the out stream from throttling the pipeline.
    with tc.tile_pool(name="opsum", bufs=3, space="PSUM") as opsum:
        for m in range(64):
            r0 = 4 * m
            ost = ostp.tile([P, 4 * W], f32)
            op = opsum.tile([P, 1024], f32)  # 2 banks; one matmul per bank
            for q in range(2):
                rhs = xch[:, r0 + 2 * q + 1:r0 + 2 * q + 3, 2:2 + W]
                nc.tensor.matmul(op[:, q * 512:(q + 1) * 512], sim[:, :],
                                 rhs, start=True, stop=True)
            # 9:7 ACT:DVE split — ACT's copies are ~20% cheaper, so give
            # it proportionally more to equalize finish times
            if (m * 7) % 16 >= 7:
                nc.scalar.copy(ost[:, :], op[:, :])
            else:
                nc.vector.tensor_copy(ost[:, :], op[:, :])
            oeng = (nc.gpsimd, nc.sync)[m % 2]
            oeng.dma_start(out=out_d[:, r0 * W:(r0 + 4) * W], in_=ost[:, :])


def _emit_conv2(nc, g2, win, w2sb, idn, b2, a2, ybufp, h2p, cpsum,
                tpsum, ptp, pts, S):
    c2 = cpsum.tile([P, 8, W2], f32, tag="cps")
    for t, (dy, dx) in enumerate(TAPS):
        rhs = win[:, dy + 1:dy + 1 + 16:2, 2 + dx:2 + dx + 2 * W2:2]
        nc.tensor.matmul(c2[:, :, :], w2sb[:, t, :], rhs,
                         start=(t == 0), stop=(t == 8))
    y2 = ybufp.tile([P, 8, W2], f16, tag="yb")
    nc.scalar.activation(y2[:, :, :], c2[:, :, :], AF.Identity,
                         bias=b2, scale=1.0)
    h2b = h2p.tile([P, 8 * W2], f16)
    nc.vector.scalar_tensor_tensor(
        out=h2b[:, :].rearrange("p (a b) -> p a b", b=W2),
        in0=y2[:, :, :], scalar=a2, in1=y2[:, :, :],
        op0=ALU.mult, op1=ALU.max)
    for q in range(4):
        tp = tpsum.tile([P, P], f16)
        nc.tensor.transpose(tp[:, :], h2b[:, q * P:(q + 1) * P], idn[:, :])
        pt = ptp.tile([P, P], f16)
        nc.scalar.copy(pt[:, :], tp[:, :])
        t = 4 * g2 + q
        # S accumulates as transposed tiles arrive (its bank is disjoint
        # from the conv/transpose banks, so interleaving groups is safe)
        nc.tensor.matmul(S[:, :], pt[:, :], pt[:, :],
                         start=(t == 0), stop=(t == 31),
                         skip_group_check=True)
        pts.append(pt)


def _prep_params(inputs):
    """Host-side: fold BN scale into diag conv weights, pack biases."""
    def fold(w, gamma, beta, mean, var):
        inv = (gamma / np.sqrt(var + EPS)).astype(np.float32)
        wf = (np.asarray(w, np.float32)[:, 0] * inv[:, None, None])
        b = (beta - mean * inv).astype(np.float32)
        return wf, b

    w1f, b1 = fold(inputs["conv1_w"], inputs["bn1_gamma"], inputs["bn1_beta"],
                   inputs["bn1_mean"], inputs["bn1_var"])
    w2f, b2 = fold(inputs["conv2_w"], inputs["bn2_gamma"], inputs["bn2_beta"],
                   inputs["bn2_mean"], inputs["bn2_var"])

    def diag(wf):
        # (P, 9, P): d[c, t, j] = wf[c, dy, dx] if j == c
        d = np.zeros((P, 9, P), np.float32)
        wt = wf.reshape(P, 9)
        d[np.arange(P), :, np.arange(P)] = wt
        return d.reshape(P, 9 * P).astype(np.float16)

    prm = np.stack([b1, np.asarray(inputs["prelu1_a"], np.float32),
                    b2, np.asarray(inputs["prelu2_a"], np.float32)],
                   axis=1).astype(np.float32)  # (P, 4)
    idn = np.eye(P, dtype=np.float16)
    return {"w1d": diag(w1f), "w2d": diag(w2f), "prm": prm, "idn": idn}


_nc_cache = {}


def get_nc(n_iters: int = 1):
    if n_iters not in _nc_cache:
        _nc_cache[n_iters] = build_nc(n_iters)
    return _nc_cache[n_iters]


def kernel(**inputs) -> np.ndarray:
    from concourse.bass_utils import run_bass_kernel_spmd

    x = np.asarray(inputs["x"], np.float32)
    B = x.shape[0]
    shared = _prep_params(inputs)
    in_maps = [dict(shared, x=np.ascontiguousarray(x[b])) for b in range(B)]
    nc = get_nc()
    res = run_bass_kernel_spmd(nc, in_maps, list(range(B)))
    out = np.stack([res.results[b]["out"].reshape(P, H, W) for b in range(B)])
    return out.astype(np.float32)
